# revision 9
# baseline (speedup 1.0000x reference)
"""MoE routing kernel for Trainium2 (Bass/Tile), 8-core data-parallel.

Problem: out = einsum('be,beo->bo', softmax(x@Wg+bg, axis=1),
                      einsum('bd,edo->beo', x, We) + be)
with B=8192, D=1024, O=1024, E=8 (all experts dense, softmax-weighted).

Strategy: shard the batch across 8 NeuronCores (1024 rows each). Each core:
  - computes gates = softmax(x@Wg + bg) on-chip (free-dim softmax),
  - transposes gates (PE transpose) to form gT for the bias term g@be,
  - for each expert: accumulates x@We[e] in PSUM (bf16 matmuls, fp32 acc),
  - combines with one fused DVE op per expert tile:
        acc = psum_e * g[:,e] + acc   (scalar_tensor_tensor)
  - acc is seeded with the bias term g@be (one K=8 matmul per tile).
Inputs are cast to bf16 host-side (x additionally pre-transposed to [D, Bs]
so it can serve as the stationary matmul operand directly).
"""
from contextlib import ExitStack

import numpy as np
import ml_dtypes

import concourse.tile as tile
import concourse.mybir as mybir
from concourse import bacc
from concourse.bass_utils import run_bass_kernel_spmd
from concourse.masks import make_identity

B, D, O, E = 8192, 1024, 1024, 8
NCORES = 8
BS = B // NCORES          # batch rows per core
P = 128                   # partition dim
NT = 512                  # matmul moving free-dim / PSUM bank width (fp32)
KC = D // P               # contraction chunks (8)
MC = BS // P              # batch-row chunks per core (8)
NCH = O // NT             # output column chunks (2)

F32 = mybir.dt.float32
BF16 = mybir.dt.bfloat16
MULT = mybir.AluOpType.mult
ADD = mybir.AluOpType.add


def _emit(nc, tc, xT, We, Wg, bg, be, out):
    ctx = ExitStack()
    with ctx:
        const = ctx.enter_context(tc.tile_pool(name="const", bufs=1))
        xp = ctx.enter_context(tc.tile_pool(name="xp", bufs=1))
        wp = ctx.enter_context(tc.tile_pool(name="wp", bufs=1))
        gp = ctx.enter_context(tc.tile_pool(name="gp", bufs=1))
        accp = ctx.enter_context(tc.tile_pool(name="accp", bufs=2))
        small = ctx.enter_context(tc.tile_pool(name="small", bufs=2))
        gps = ctx.enter_context(tc.tile_pool(name="gps", bufs=2, space="PSUM"))
        tps = ctx.enter_context(tc.tile_pool(name="tps", bufs=1, space="PSUM"))
        bps = ctx.enter_context(tc.tile_pool(name="bps", bufs=2, space="PSUM"))
        eps = ctx.enter_context(tc.tile_pool(name="eps", bufs=3, space="PSUM"))

        # ---- loads ----
        # DMA emission order = queue fill order: small gate constants first,
        # then xT (gate matmuls need every k-chunk), then expert-0 weights so
        # the expert stream can start, then the remaining experts.
        # Batched DMAs: each dma_start costs ~600ns of sequencer issue time
        # and the 16 SDMA engines drain queued packets FIFO — so the loads
        # the kernel needs first (xT, then Wg) are issued first, split
        # across both HWDGE queues (scalar + sync); the big We stream after.
        wg_all = const.tile([P, KC * E], BF16, name="wg_all")
        nc.scalar.dma_start(
            wg_all[:].rearrange("p (k e) -> p k e", k=KC),
            Wg.rearrange("(k p) e -> p k e", p=P))

        bg_sb = const.tile([1, E], F32, name="bg_sb")
        nc.scalar.dma_start(bg_sb[:], bg)
        be_sb = const.tile([E, O], BF16, name="be_sb")
        nc.scalar.dma_start(be_sb[:], be)

        xt_all = xp.tile([P, KC * BS], BF16, name="xt_all")
        KH = KC // 2
        for h, eng in ((0, nc.scalar), (1, nc.sync)):
            eng.dma_start(
                xt_all[:, h * KH * BS:(h + 1) * KH * BS]
                .rearrange("p (k b) -> p k b", k=KH),
                xT[h * KH * P:(h + 1) * KH * P, :]
                .rearrange("(k p) b -> p k b", p=P))

        ones_sb = const.tile([1, P], F32, name="ones_sb")
        nc.vector.memset(ones_sb[:], 1.0)
        ident = const.tile([P, P], F32, name="ident")
        make_identity(nc, ident[:])

        def xt(k, ms):
            return xt_all[:, k * BS + ms.start:k * BS + ms.stop]

        def wg(k):
            return wg_all[:, k * E:(k + 1) * E]

        # We: 4 quarter-loads per expert (2 k-chunks each) on the sync queue
        we_all = []
        for e in range(E):
            t = wp.tile([P, KC * O], BF16, name=f"we{e}", tag=f"we{e}")
            for q in range(4):
                nc.sync.dma_start(
                    t[:, q * 2 * O:(q + 1) * 2 * O]
                    .rearrange("p (k o) -> p k o", k=2),
                    We[e, q * 2 * P:(q + 1) * 2 * P, :]
                    .rearrange("(k p) o -> p k o", p=P))
            we_all.append(t)

        def we(e, k, ns):
            return we_all[e][:, k * O + ns.start:k * O + ns.stop]

        # ---- gates: softmax(x @ Wg + bg) ----
        gates_sb = []
        gT_all = gp.tile([E, BS], BF16, name="gT_all")
        for m in range(MC):
            ms = slice(m * P, (m + 1) * P)
            pg = gps.tile([P, E], F32, name="pg", tag="pg")
            for k in range(KC):
                nc.tensor.matmul(pg[:], xt(k, ms), wg(k),
                                 start=(k == 0), stop=False)
            nc.tensor.matmul(pg[:], ones_sb[:], bg_sb[:], start=False, stop=True)

            rmax = small.tile([P, 1], F32, name="rmax", tag="rmax")
            nc.vector.tensor_reduce(rmax[:], pg[:], axis=mybir.AxisListType.X,
                                    op=mybir.AluOpType.max)
            nmax = small.tile([P, 1], F32, name="nmax", tag="nmax")
            nc.vector.tensor_scalar_mul(nmax[:], rmax[:], -1.0)

            g = gp.tile([P, E], F32, name=f"g{m}", tag=f"g{m}")
            den = small.tile([P, 1], F32, name="den", tag="den")
            nc.scalar.activation(g[:], pg[:], mybir.ActivationFunctionType.Exp,
                                 bias=nmax[:], scale=1.0, accum_out=den[:])
            rden = small.tile([P, 1], F32, name="rden", tag="rden")
            nc.vector.reciprocal(rden[:], den[:])
            nc.vector.tensor_scalar_mul(g[:], g[:], rden[:])
            gates_sb.append(g)

            pt = tps.tile([E, P], F32, name="pt", tag="pt")
            nc.tensor.transpose(pt[:], g[:], ident[:])
            nc.scalar.copy(gT_all[:, ms], pt[:])

        # ---- experts + combine ----
        for n in range(NCH):
            ns = slice(n * NT, (n + 1) * NT)
            accs = []
            for m in range(MC):
                ms = slice(m * P, (m + 1) * P)
                pb = bps.tile([P, NT], F32, name="pb", tag="pb")
                nc.tensor.matmul(pb[:], gT_all[:, ms], be_sb[:, ns],
                                 start=True, stop=True)
                acc = accp.tile([P, NT], F32, name=f"acc{m}", tag=f"acc{m}")
                nc.scalar.copy(acc[:], pb[:])
                accs.append(acc)
            for e in range(E):
                for m in range(MC):
                    ms = slice(m * P, (m + 1) * P)
                    pe = eps.tile([P, NT], F32, name="pe", tag="pe")
                    for k in range(KC):
                        nc.tensor.matmul(pe[:], xt(k, ms), we(e, k, ns),
                                         start=(k == 0), stop=(k == KC - 1))
                    nc.vector.scalar_tensor_tensor(
                        accs[m][:], pe[:], gates_sb[m][:, e:e + 1], accs[m][:],
                        MULT, ADD)
            for m in range(MC):
                nc.scalar.dma_start(out[m * P:(m + 1) * P, ns], accs[m][:])


_NC_CACHE = {}


def _build():
    if "nc" in _NC_CACHE:
        return _NC_CACHE["nc"]
    nc = bacc.Bacc("TRN2", target_bir_lowering=False, debug=False,
                   num_devices=NCORES)
    xT = nc.dram_tensor("xT", [D, BS], BF16, kind="ExternalInput").ap()
    We_t = nc.dram_tensor("We", [E, D, O], BF16, kind="ExternalInput").ap()
    Wg_t = nc.dram_tensor("Wg", [D, E], BF16, kind="ExternalInput").ap()
    bg_t = nc.dram_tensor("bg", [1, E], F32, kind="ExternalInput").ap()
    be_t = nc.dram_tensor("be", [E, O], BF16, kind="ExternalInput").ap()
    out = nc.dram_tensor("out", [BS, O], F32, kind="ExternalOutput").ap()
    with tile.TileContext(nc) as tc:
        _emit(nc, tc, xT, We_t, Wg_t, bg_t, be_t, out)
    nc.compile()
    _NC_CACHE["nc"] = nc
    return nc


def _in_maps(x, Wg, bg, We, be):
    bf = ml_dtypes.bfloat16
    x = np.asarray(x, dtype=np.float32)
    We_bf = np.asarray(We, dtype=np.float32).astype(bf)
    Wg_bf = np.asarray(Wg, dtype=np.float32).astype(bf)
    be_bf = np.asarray(be, dtype=np.float32).astype(bf)
    bg32 = np.asarray(bg, dtype=np.float32).reshape(1, E)
    maps = []
    for c in range(NCORES):
        xT = np.ascontiguousarray(x[c * BS:(c + 1) * BS].T).astype(bf)
        maps.append({"xT": xT, "We": We_bf, "Wg": Wg_bf,
                     "bg": bg32, "be": be_bf})
    return maps


def run(x, Wg, bg, We, be, **spmd_kwargs):
    nc = _build()
    maps = _in_maps(x, Wg, bg, We, be)
    res = run_bass_kernel_spmd(nc, maps, core_ids=list(range(NCORES)),
                               **spmd_kwargs)
    out = np.concatenate([res.results[c]["out"] for c in range(NCORES)],
                         axis=0)
    return out, res


def kernel(x, Wg, bg, We, be):
    out, _ = run(x, Wg, bg, We, be)
    return out


# revision 11
# speedup vs baseline: 1.0096x; 1.0096x over previous
"""MoE routing kernel for Trainium2 (Bass/Tile), 8-core data-parallel.

Problem: out = einsum('be,beo->bo', softmax(x@Wg+bg, axis=1),
                      einsum('bd,edo->beo', x, We) + be)
with B=8192, D=1024, O=1024, E=8 (all experts dense, softmax-weighted).

Strategy: shard the batch across 8 NeuronCores (1024 rows each). Each core:
  - computes gates = softmax(x@Wg + bg) on-chip (free-dim softmax),
  - transposes gates (PE transpose) to form gT for the bias term g@be,
  - for each expert: accumulates x@We[e] in PSUM (bf16 matmuls, fp32 acc),
  - combines with one fused DVE op per expert tile:
        acc = psum_e * g[:,e] + acc   (scalar_tensor_tensor)
  - acc is seeded with the bias term g@be (one K=8 matmul per tile).
Inputs are cast to bf16 host-side (x additionally pre-transposed to [D, Bs]
so it can serve as the stationary matmul operand directly).
"""
from contextlib import ExitStack

import numpy as np
import ml_dtypes

import concourse.tile as tile
import concourse.mybir as mybir
from concourse import bacc
from concourse.bass_utils import run_bass_kernel_spmd
from concourse.masks import make_identity

B, D, O, E = 8192, 1024, 1024, 8
NCORES = 8
BS = B // NCORES          # batch rows per core
P = 128                   # partition dim
NT = 512                  # matmul moving free-dim / PSUM bank width (fp32)
KC = D // P               # contraction chunks (8)
MC = BS // P              # batch-row chunks per core (8)
NCH = O // NT             # output column chunks (2)

F32 = mybir.dt.float32
BF16 = mybir.dt.bfloat16
MULT = mybir.AluOpType.mult
ADD = mybir.AluOpType.add


def _emit(nc, tc, xT, We, Wg, bg, be, out):
    ctx = ExitStack()
    with ctx:
        const = ctx.enter_context(tc.tile_pool(name="const", bufs=1))
        xp = ctx.enter_context(tc.tile_pool(name="xp", bufs=1))
        wp = ctx.enter_context(tc.tile_pool(name="wp", bufs=1))
        gp = ctx.enter_context(tc.tile_pool(name="gp", bufs=1))
        accp = ctx.enter_context(tc.tile_pool(name="accp", bufs=2))
        small = ctx.enter_context(tc.tile_pool(name="small", bufs=2))
        gps = ctx.enter_context(tc.tile_pool(name="gps", bufs=2, space="PSUM"))
        tps = ctx.enter_context(tc.tile_pool(name="tps", bufs=1, space="PSUM"))
        bps = ctx.enter_context(tc.tile_pool(name="bps", bufs=2, space="PSUM"))
        eps = ctx.enter_context(tc.tile_pool(name="eps", bufs=3, space="PSUM"))

        # ---- loads ----
        # DMA emission order = queue fill order: small gate constants first,
        # then xT (gate matmuls need every k-chunk), then expert-0 weights so
        # the expert stream can start, then the remaining experts.
        # Batched DMAs: each dma_start costs ~600ns of sequencer issue time
        # and the 16 SDMA engines drain queued packets FIFO — so the loads
        # the kernel needs first (xT, then Wg) are issued first, split
        # across both HWDGE queues (scalar + sync); the big We stream after.
        wg_all = const.tile([P, KC * E], BF16, name="wg_all")
        nc.scalar.dma_start(
            wg_all[:].rearrange("p (k e) -> p k e", k=KC),
            Wg.rearrange("(k p) e -> p k e", p=P))

        bg_sb = const.tile([1, E], F32, name="bg_sb")
        nc.scalar.dma_start(bg_sb[:], bg)
        be_sb = const.tile([E, O], BF16, name="be_sb")
        nc.scalar.dma_start(be_sb[:], be)

        # xT as 8 per-chunk DMAs split over both queues, so gate matmul k can
        # start as soon as chunk k lands
        xt_all = xp.tile([P, KC * BS], BF16, name="xt_all")
        for k in range(KC):
            eng = nc.scalar if k % 2 == 0 else nc.sync
            eng.dma_start(xt_all[:, k * BS:(k + 1) * BS],
                          xT[k * P:(k + 1) * P, :])

        ones_sb = const.tile([1, P], F32, name="ones_sb")
        nc.vector.memset(ones_sb[:], 1.0)
        ident = const.tile([P, P], F32, name="ident")
        make_identity(nc, ident[:])

        def xt(k, ms):
            return xt_all[:, k * BS + ms.start:k * BS + ms.stop]

        def wg(k):
            return wg_all[:, k * E:(k + 1) * E]

        # We: 4 quarter-loads per expert (2 k-chunks each) on the sync queue
        we_all = []
        for e in range(E):
            t = wp.tile([P, KC * O], BF16, name=f"we{e}", tag=f"we{e}")
            for q in range(4):
                nc.sync.dma_start(
                    t[:, q * 2 * O:(q + 1) * 2 * O]
                    .rearrange("p (k o) -> p k o", k=2),
                    We[e, q * 2 * P:(q + 1) * 2 * P, :]
                    .rearrange("(k p) o -> p k o", p=P))
            we_all.append(t)

        def we(e, k, ns):
            return we_all[e][:, k * O + ns.start:k * O + ns.stop]

        # ---- PE warm-up ----
        # HAM keeps the PE clock-gated at 1.2 GHz until ~3.4us of sustained
        # matmul activity. Burn throwaway matmuls on a zero tile while the
        # input DMAs are in flight so the real stream runs at 2.4 GHz.
        warm_sb = const.tile([P, NT], BF16, name="warm_sb")
        nc.vector.memset(warm_sb[:], 0.0)
        for w in range(14):
            pwu = bps.tile([P, NT], F32, name="pwu", tag="pb")
            nc.tensor.matmul(pwu[:], warm_sb[:, :P], warm_sb[:],
                             start=True, stop=True)

        # ---- gates: softmax(x @ Wg + bg) ----
        gates_sb = []
        gT_all = gp.tile([E, BS], BF16, name="gT_all")
        for m in range(MC):
            ms = slice(m * P, (m + 1) * P)
            pg = gps.tile([P, E], F32, name="pg", tag="pg")
            for k in range(KC):
                nc.tensor.matmul(pg[:], xt(k, ms), wg(k),
                                 start=(k == 0), stop=False)
            nc.tensor.matmul(pg[:], ones_sb[:], bg_sb[:], start=False, stop=True)

            rmax = small.tile([P, 1], F32, name="rmax", tag="rmax")
            nc.vector.tensor_reduce(rmax[:], pg[:], axis=mybir.AxisListType.X,
                                    op=mybir.AluOpType.max)
            nmax = small.tile([P, 1], F32, name="nmax", tag="nmax")
            nc.vector.tensor_scalar_mul(nmax[:], rmax[:], -1.0)

            g = gp.tile([P, E], F32, name=f"g{m}", tag=f"g{m}")
            den = small.tile([P, 1], F32, name="den", tag="den")
            nc.scalar.activation(g[:], pg[:], mybir.ActivationFunctionType.Exp,
                                 bias=nmax[:], scale=1.0, accum_out=den[:])
            rden = small.tile([P, 1], F32, name="rden", tag="rden")
            nc.vector.reciprocal(rden[:], den[:])
            nc.vector.tensor_scalar_mul(g[:], g[:], rden[:])
            gates_sb.append(g)

            pt = tps.tile([E, P], F32, name="pt", tag="pt")
            nc.tensor.transpose(pt[:], g[:], ident[:])
            nc.scalar.copy(gT_all[:, ms], pt[:])

        # ---- experts + combine ----
        for n in range(NCH):
            ns = slice(n * NT, (n + 1) * NT)
            accs = []
            for m in range(MC):
                ms = slice(m * P, (m + 1) * P)
                pb = bps.tile([P, NT], F32, name="pb", tag="pb")
                nc.tensor.matmul(pb[:], gT_all[:, ms], be_sb[:, ns],
                                 start=True, stop=True)
                acc = accp.tile([P, NT], F32, name=f"acc{m}", tag=f"acc{m}")
                nc.scalar.copy(acc[:], pb[:])
                accs.append(acc)
            for e in range(E):
                for m in range(MC):
                    ms = slice(m * P, (m + 1) * P)
                    pe = eps.tile([P, NT], F32, name="pe", tag="pe")
                    for k in range(KC):
                        nc.tensor.matmul(pe[:], xt(k, ms), we(e, k, ns),
                                         start=(k == 0), stop=(k == KC - 1))
                    nc.vector.scalar_tensor_tensor(
                        accs[m][:], pe[:], gates_sb[m][:, e:e + 1], accs[m][:],
                        MULT, ADD)
            for m in range(MC):
                nc.scalar.dma_start(out[m * P:(m + 1) * P, ns], accs[m][:])


_NC_CACHE = {}


def _build():
    if "nc" in _NC_CACHE:
        return _NC_CACHE["nc"]
    nc = bacc.Bacc("TRN2", target_bir_lowering=False, debug=False,
                   num_devices=NCORES)
    xT = nc.dram_tensor("xT", [D, BS], BF16, kind="ExternalInput").ap()
    We_t = nc.dram_tensor("We", [E, D, O], BF16, kind="ExternalInput").ap()
    Wg_t = nc.dram_tensor("Wg", [D, E], BF16, kind="ExternalInput").ap()
    bg_t = nc.dram_tensor("bg", [1, E], F32, kind="ExternalInput").ap()
    be_t = nc.dram_tensor("be", [E, O], BF16, kind="ExternalInput").ap()
    out = nc.dram_tensor("out", [BS, O], F32, kind="ExternalOutput").ap()
    with tile.TileContext(nc) as tc:
        _emit(nc, tc, xT, We_t, Wg_t, bg_t, be_t, out)
    nc.compile()
    _NC_CACHE["nc"] = nc
    return nc


def _in_maps(x, Wg, bg, We, be):
    bf = ml_dtypes.bfloat16
    x = np.asarray(x, dtype=np.float32)
    We_bf = np.asarray(We, dtype=np.float32).astype(bf)
    Wg_bf = np.asarray(Wg, dtype=np.float32).astype(bf)
    be_bf = np.asarray(be, dtype=np.float32).astype(bf)
    bg32 = np.asarray(bg, dtype=np.float32).reshape(1, E)
    maps = []
    for c in range(NCORES):
        xT = np.ascontiguousarray(x[c * BS:(c + 1) * BS].T).astype(bf)
        maps.append({"xT": xT, "We": We_bf, "Wg": Wg_bf,
                     "bg": bg32, "be": be_bf})
    return maps


def run(x, Wg, bg, We, be, **spmd_kwargs):
    nc = _build()
    maps = _in_maps(x, Wg, bg, We, be)
    res = run_bass_kernel_spmd(nc, maps, core_ids=list(range(NCORES)),
                               **spmd_kwargs)
    out = np.concatenate([res.results[c]["out"] for c in range(NCORES)],
                         axis=0)
    return out, res


def kernel(x, Wg, bg, We, be):
    out, _ = run(x, Wg, bg, We, be)
    return out


# revision 14
# speedup vs baseline: 1.0199x; 1.0102x over previous
"""MoE routing kernel for Trainium2 (Bass/Tile), 8-core data-parallel.

Problem: out = einsum('be,beo->bo', softmax(x@Wg+bg, axis=1),
                      einsum('bd,edo->beo', x, We) + be)
with B=8192, D=1024, O=1024, E=8 (all experts dense, softmax-weighted).

Strategy: shard the batch across 8 NeuronCores (1024 rows each). Each core:
  - computes gates = softmax(x@Wg + bg) on-chip (free-dim softmax),
  - transposes gates (PE transpose) to form gT for the bias term g@be,
  - for each expert: accumulates x@We[e] in PSUM (bf16 matmuls, fp32 acc),
  - combines with one fused DVE op per expert tile:
        acc = psum_e * g[:,e] + acc   (scalar_tensor_tensor)
  - acc is seeded with the bias term g@be (one K=8 matmul per tile).
Inputs are cast to bf16 host-side (x additionally pre-transposed to [D, Bs]
so it can serve as the stationary matmul operand directly).
"""
from contextlib import ExitStack

import numpy as np
import ml_dtypes

import concourse.tile as tile
import concourse.mybir as mybir
from concourse import bacc
from concourse.bass_utils import run_bass_kernel_spmd
from concourse.masks import make_identity

B, D, O, E = 8192, 1024, 1024, 8
NCORES = 8
BS = B // NCORES          # batch rows per core
P = 128                   # partition dim
NT = 512                  # matmul moving free-dim / PSUM bank width (fp32)
KC = D // P               # contraction chunks (8)
MC = BS // P              # batch-row chunks per core (8)
NCH = O // NT             # output column chunks (2)

F32 = mybir.dt.float32
BF16 = mybir.dt.bfloat16
MULT = mybir.AluOpType.mult
ADD = mybir.AluOpType.add


def _emit(nc, tc, xT, We, Wg, bg, be, out):
    ctx = ExitStack()
    with ctx:
        const = ctx.enter_context(tc.tile_pool(name="const", bufs=1))
        xp = ctx.enter_context(tc.tile_pool(name="xp", bufs=1))
        wp = ctx.enter_context(tc.tile_pool(name="wp", bufs=1))
        gp = ctx.enter_context(tc.tile_pool(name="gp", bufs=1))
        accp = ctx.enter_context(tc.tile_pool(name="accp", bufs=2))
        small = ctx.enter_context(tc.tile_pool(name="small", bufs=2))
        gps = ctx.enter_context(tc.tile_pool(name="gps", bufs=2, space="PSUM"))
        tps = ctx.enter_context(tc.tile_pool(name="tps", bufs=1, space="PSUM"))
        bps = ctx.enter_context(tc.tile_pool(name="bps", bufs=2, space="PSUM"))
        eps = ctx.enter_context(tc.tile_pool(name="eps", bufs=3, space="PSUM"))

        # ---- loads ----
        # DMA emission order = queue fill order: small gate constants first,
        # then xT (gate matmuls need every k-chunk), then expert-0 weights so
        # the expert stream can start, then the remaining experts.
        # Batched DMAs: each dma_start costs ~600ns of sequencer issue time
        # and the 16 SDMA engines drain queued packets FIFO — so the loads
        # the kernel needs first (xT, then Wg) are issued first, split
        # across both HWDGE queues (scalar + sync); the big We stream after.
        wg_all = const.tile([P, KC * E], BF16, name="wg_all")
        nc.scalar.dma_start(
            wg_all[:].rearrange("p (k e) -> p k e", k=KC),
            Wg.rearrange("(k p) e -> p k e", p=P))

        bg_sb = const.tile([1, E], F32, name="bg_sb")
        nc.scalar.dma_start(bg_sb[:], bg)
        be_sb = const.tile([E, O], BF16, name="be_sb")
        nc.scalar.dma_start(be_sb[:], be)

        # xT as 8 per-chunk DMAs split over both queues, so gate matmul k can
        # start as soon as chunk k lands
        xt_all = xp.tile([P, KC * BS], BF16, name="xt_all")
        for k in range(KC):
            eng = nc.scalar if k % 2 == 0 else nc.sync
            eng.dma_start(xt_all[:, k * BS:(k + 1) * BS],
                          xT[k * P:(k + 1) * P, :])

        ones_sb = const.tile([1, P], F32, name="ones_sb")
        nc.vector.memset(ones_sb[:], 1.0)
        ident = const.tile([P, P], F32, name="ident")
        make_identity(nc, ident[:])

        def xt(k, ms):
            return xt_all[:, k * BS + ms.start:k * BS + ms.stop]

        def wg(k):
            return wg_all[:, k * E:(k + 1) * E]

        # We: 4 quarter-loads per expert (2 k-chunks each) on the sync queue
        we_all = []
        for e in range(E):
            t = wp.tile([P, KC * O], BF16, name=f"we{e}", tag=f"we{e}")
            for q in range(4):
                nc.sync.dma_start(
                    t[:, q * 2 * O:(q + 1) * 2 * O]
                    .rearrange("p (k o) -> p k o", k=2),
                    We[e, q * 2 * P:(q + 1) * 2 * P, :]
                    .rearrange("(k p) o -> p k o", p=P))
            we_all.append(t)

        def we(e, k, ns):
            return we_all[e][:, k * O + ns.start:k * O + ns.stop]

        # ---- PE warm-up ----
        # HAM keeps the PE clock-gated at 1.2 GHz until ~3.4us of sustained
        # matmul activity. Burn throwaway matmuls on a zero tile while the
        # input DMAs are in flight so the real stream runs at 2.4 GHz.
        warm_sb = const.tile([P, NT], BF16, name="warm_sb")
        nc.vector.memset(warm_sb[:], 0.0)

        def warmup(n):
            for _ in range(n):
                pwu = bps.tile([P, NT], F32, name="pwu", tag="pb")
                nc.tensor.matmul(pwu[:], warm_sb[:, :P], warm_sb[:],
                                 start=True, stop=True)

        warmup(14)

        # ---- gates: softmax(x @ Wg + bg) ----
        gates_sb = []
        gT_all = gp.tile([E, BS], BF16, name="gT_all")
        for m in range(MC):
            ms = slice(m * P, (m + 1) * P)
            pg = gps.tile([P, E], F32, name="pg", tag="pg")
            for k in range(KC):
                nc.tensor.matmul(pg[:], xt(k, ms), wg(k),
                                 start=(k == 0), stop=False)
                if m == 0:
                    # keep PE duty high while xT chunks trickle in, else HAM
                    # re-throttles the clock during this sparse stretch
                    warmup(2)
            nc.tensor.matmul(pg[:], ones_sb[:], bg_sb[:], start=False, stop=True)

            rmax = small.tile([P, 1], F32, name="rmax", tag="rmax")
            nc.vector.tensor_reduce(rmax[:], pg[:], axis=mybir.AxisListType.X,
                                    op=mybir.AluOpType.max)
            nmax = small.tile([P, 1], F32, name="nmax", tag="nmax")
            nc.vector.tensor_scalar_mul(nmax[:], rmax[:], -1.0)

            g = gp.tile([P, E], F32, name=f"g{m}", tag=f"g{m}")
            den = small.tile([P, 1], F32, name="den", tag="den")
            nc.scalar.activation(g[:], pg[:], mybir.ActivationFunctionType.Exp,
                                 bias=nmax[:], scale=1.0, accum_out=den[:])
            rden = small.tile([P, 1], F32, name="rden", tag="rden")
            nc.vector.reciprocal(rden[:], den[:])
            nc.vector.tensor_scalar_mul(g[:], g[:], rden[:])
            gates_sb.append(g)

            warmup(1)
            pt = tps.tile([E, P], F32, name="pt", tag="pt")
            nc.tensor.transpose(pt[:], g[:], ident[:])
            nc.scalar.copy(gT_all[:, ms], pt[:])
        warmup(4)

        # ---- experts + combine ----
        for n in range(NCH):
            ns = slice(n * NT, (n + 1) * NT)
            accs = []
            for m in range(MC):
                ms = slice(m * P, (m + 1) * P)
                pb = bps.tile([P, NT], F32, name="pb", tag="pb")
                nc.tensor.matmul(pb[:], gT_all[:, ms], be_sb[:, ns],
                                 start=True, stop=True)
                acc = accp.tile([P, NT], F32, name=f"acc{m}", tag=f"acc{m}")
                nc.scalar.copy(acc[:], pb[:])
                accs.append(acc)
            for e in range(E):
                for m in range(MC):
                    ms = slice(m * P, (m + 1) * P)
                    pe = eps.tile([P, NT], F32, name="pe", tag="pe")
                    for k in range(KC):
                        nc.tensor.matmul(pe[:], xt(k, ms), we(e, k, ns),
                                         start=(k == 0), stop=(k == KC - 1))
                    nc.vector.scalar_tensor_tensor(
                        accs[m][:], pe[:], gates_sb[m][:, e:e + 1], accs[m][:],
                        MULT, ADD)
            for m in range(MC):
                nc.scalar.dma_start(out[m * P:(m + 1) * P, ns], accs[m][:])


_NC_CACHE = {}


def _build():
    if "nc" in _NC_CACHE:
        return _NC_CACHE["nc"]
    nc = bacc.Bacc("TRN2", target_bir_lowering=False, debug=False,
                   num_devices=NCORES)
    xT = nc.dram_tensor("xT", [D, BS], BF16, kind="ExternalInput").ap()
    We_t = nc.dram_tensor("We", [E, D, O], BF16, kind="ExternalInput").ap()
    Wg_t = nc.dram_tensor("Wg", [D, E], BF16, kind="ExternalInput").ap()
    bg_t = nc.dram_tensor("bg", [1, E], F32, kind="ExternalInput").ap()
    be_t = nc.dram_tensor("be", [E, O], BF16, kind="ExternalInput").ap()
    out = nc.dram_tensor("out", [BS, O], F32, kind="ExternalOutput").ap()
    with tile.TileContext(nc) as tc:
        _emit(nc, tc, xT, We_t, Wg_t, bg_t, be_t, out)
    nc.compile()
    _NC_CACHE["nc"] = nc
    return nc


def _in_maps(x, Wg, bg, We, be):
    bf = ml_dtypes.bfloat16
    x = np.asarray(x, dtype=np.float32)
    We_bf = np.asarray(We, dtype=np.float32).astype(bf)
    Wg_bf = np.asarray(Wg, dtype=np.float32).astype(bf)
    be_bf = np.asarray(be, dtype=np.float32).astype(bf)
    bg32 = np.asarray(bg, dtype=np.float32).reshape(1, E)
    maps = []
    for c in range(NCORES):
        xT = np.ascontiguousarray(x[c * BS:(c + 1) * BS].T).astype(bf)
        maps.append({"xT": xT, "We": We_bf, "Wg": Wg_bf,
                     "bg": bg32, "be": be_bf})
    return maps


def run(x, Wg, bg, We, be, **spmd_kwargs):
    nc = _build()
    maps = _in_maps(x, Wg, bg, We, be)
    res = run_bass_kernel_spmd(nc, maps, core_ids=list(range(NCORES)),
                               **spmd_kwargs)
    out = np.concatenate([res.results[c]["out"] for c in range(NCORES)],
                         axis=0)
    return out, res


def kernel(x, Wg, bg, We, be):
    out, _ = run(x, Wg, bg, We, be)
    return out
